# revision 4
# baseline (speedup 1.0000x reference)
"""Trainium2 Bass kernel v7 = v6 + prewarm/memset fixes from v6's trace.

v6 measured 19.4 us. Its trace showed the ACT Copy-table prewarm was
gated on the 3.5 us ebuf memset (s_m), putting the 1.28 us table load
back on the critical path, and the memset itself (no fast mode at fp16)
delayed DVE's first rot past input-arrival. Fixes: dummy gets its own
semaphore so the table loads at ~7 us during the input DMA, and the
memset runs on a bitcast-fp32 view (half the elements).

v5's NTFF trace showed the output was HWDGE descriptor-generation bound:
4096 row-descriptors at ~1.65 ns each ≈ 7 us on one ring, with per-packet
floors ~10-17 ns swamping the byte savings of fp16.

v6 kills the descriptor count: with fp16 output, each written channel
(2c+r) is ONE contiguous span. The key identity: in a row-major image,
shifting the flat span start by ph*64 + pw shifts rows by ph AND columns
by pw at once. So the kernel writes E'[c] - a fixed 64x64 fp16 image with
values at even (row, col) positions, zeros elsewhere (pre-memset) - and
the whole per-sample (r, ph, pw) selection is a single dynamic flat
offset od = r*4096 + ph*64 + pw on the (c, two*4096) view. Span lengths
are tightened to 128k-65 so the largest end index is exactly 8191 (last
nonzero element), and inter-chunk/channel gaps land on positions that are
zero by the runtime's zero-init contract.

Per chunk (k a-rows): ACT rot (k*32 elems), DVE fused select (k*32), one
output DMA of 128 descriptors (~2 KiB each). Total 512 descriptors vs
v5's 4096.
"""

import numpy as np

B, C, H, W = 8, 128, 32, 32
OC, OH, OW = 2 * C, 2 * H, 2 * W
N_CORES = 8

CHUNKS = (4, 8, 8, 12)
_compiled = {}


def _build_bass(chunks=None):
    from contextlib import ExitStack

    import concourse.bass as bass
    import concourse.mybir as mybir

    chunks = chunks or CHUNKS

    fp32 = mybir.dt.float32
    fp16 = mybir.dt.float16
    orig_aeb = bass.Bass.all_engine_barrier
    bass.Bass.all_engine_barrier = lambda self, **kw: None
    try:
        nc = bass.Bass(enable_partition_id=False)
    finally:
        bass.Bass.all_engine_barrier = orig_aeb

    Copy = mybir.ActivationFunctionType.Copy
    SP = mybir.EngineType.SP

    hdr = nc.dram_tensor("hdr", (C, 8), fp32, kind="ExternalInput")
    xh = nc.dram_tensor("xh", (C, H * W), fp16, kind="ExternalInput")
    out = nc.dram_tensor("out", (OC, OH * OW), fp16, kind="ExternalOutput")

    ctx = ExitStack()
    with ctx:
        hbuf = ctx.enter_context(nc.sbuf_tensor([C, 8], fp32))
        xbuf = ctx.enter_context(nc.sbuf_tensor([C, H * W], fp16))
        ubuf = ctx.enter_context(nc.sbuf_tensor([C, H * W], fp16))
        ebuf = ctx.enter_context(nc.sbuf_tensor([C, OH * OW], fp16))  # 64x64
        dummy = ctx.enter_context(nc.sbuf_tensor([C, 2], fp32))

        s_d = nc.alloc_semaphore("s_d")  # dummy ready (table prewarm)
        s_h = nc.alloc_semaphore("s_h")
        s_x = nc.alloc_semaphore("s_x")
        s_m = nc.alloc_semaphore("s_m")
        s_r0 = nc.alloc_semaphore("s_r0")
        s_r = nc.alloc_semaphore("s_r")
        s_v = nc.alloc_semaphore("s_v")
        s_out = nc.alloc_semaphore("s_out")

        x3 = xbuf[:].rearrange("p (a b) -> p a b", b=W)
        u3 = ubuf[:].rearrange("p (a b) -> p a b", b=W)
        e3 = ebuf[:].rearrange("p (oh ow) -> p oh ow", ow=OW)  # (c, 64, 64)
        # channel pair (2c, 2c+1) flattened: od = r*4096 + ph*64 + pw picks
        # the start; spans tile exactly at 1024-elem strides per chunk
        outv = out[:].rearrange("(c two) m -> c (two m)", two=2)  # (128, 8192)
        wm = hbuf[:, 0:2]  # [m1, m0]
        qbits = hbuf[:, 2:8].bitcast(mybir.dt.int32)

        # ---- T0: header on SP ring, x on ACT ring ----
        nc.sync.dma_start(hbuf[:], hdr[:]).then_inc(s_h, 16)
        nc.scalar.dma_start(xbuf[:], xh[:]).then_inc(s_x, 16)

        # ---- T0: DVE zero-fills E' + dummy; ACT prewarms Copy table ----
        nc.vector.memset(dummy[:, :], 0.0).then_inc(s_d, 1)
        nc.vector.memset(ebuf[:].bitcast(fp32), 0.0).then_inc(s_m, 1)
        nc.scalar.wait_ge(s_d, 1)
        nc.scalar.activation(dummy[:, 1:2], dummy[:, 0:1], Copy, scale=0.0)

        # ---- SP: one offset register, pre-dispatch output DMAs ----
        nc.sync.wait_ge(s_h, 16)
        od0 = nc.values_load(
            qbits[0:1, 0:1],
            engines=[SP],
            min_val=0,
            max_val=OH * OW + OW + 1,  # 4161
            skip_runtime_bounds_check=True,
        )
        nc.sync.wait_ge(s_m, 1)
        a0 = 0
        for j, n in enumerate(chunks):
            odj = od0 if j == 0 else nc.sync.compute_val(od0 + 128 * a0)
            span = 128 * n - 65  # through the last nonzero element
            nc.sync.dma_start(
                outv[:, bass.ds(odj, span)],
                ebuf[:, 128 * a0 : 128 * a0 + span],
            )._wait_ge(s_v, j + 1).then_inc(s_out, 16)
            a0 += n

        # ---- DVE: chunk-0 rot locally, then fused select per chunk ----
        nc.vector.wait_ge(s_h, 16)
        nc.vector.wait_ge(s_x, 16)
        n0 = chunks[0]
        nc.vector.tensor_scalar_mul(
            u3[:, 0:1, :], x3[:, :, 0:1].transpose([0, 2, 1]), wm[:, 0:1]
        )
        nc.vector.tensor_scalar_mul(
            u3[:, 1:n0, :],
            x3[:, :, W - 1 : W - n0 : -1].transpose([0, 2, 1]),
            wm[:, 0:1],
        ).then_inc(s_r0, 1)

        # ---- ACT: rots for chunks 1.. ----
        nc.scalar.wait_ge(s_h, 16)
        nc.scalar.wait_ge(s_x, 16)
        a0 = n0
        for j, n in enumerate(chunks[1:], start=1):
            nc.scalar.activation(
                u3[:, a0 : a0 + n, :],
                x3[:, :, W - a0 : W - a0 - n : -1].transpose([0, 2, 1]),
                Copy,
                scale=wm[:, 0:1],
            ).then_inc(s_r, 1)
            a0 += n

        a0 = 0
        for j, n in enumerate(chunks):
            nc.vector.wait_ge(s_r0, 1) if j == 0 else nc.vector.wait_ge(s_r, j)
            nc.vector.scalar_tensor_tensor(
                e3[:, 2 * a0 : 2 * (a0 + n) : 2, 0::2],
                x3[:, a0 : a0 + n, :],
                wm[:, 1:2],
                u3[:, a0 : a0 + n, :],
                op0=mybir.AluOpType.mult,
                op1=mybir.AluOpType.add,
            ).then_inc(s_v, 1)
            a0 += n

        nc.sync.wait_ge(s_out, 16 * len(chunks))
    return nc


def _get_bass():
    if "nc" not in _compiled:
        _compiled["nc"] = _build_bass()
    return _compiled["nc"]


def _make_in_maps(x, p, chunks=None):
    x = np.asarray(x, dtype=np.float32)
    p = np.asarray(p)
    xh_all = x.reshape(B, C, H * W).astype(np.float16)
    in_maps = []
    for i in range(B):
        ph, pw, r = int(p[i, 0]), int(p[i, 1]), int(p[i, 2])
        assert r in (0, 1) and ph in (0, 1) and pw in (0, 1)
        w = np.zeros(8, np.float32)
        w[0] = 1.0 if r == 1 else 0.0  # m1
        w[1] = 1.0 if r == 0 else 0.0  # m0
        w[2] = np.int32(r * 4096 + ph * 64 + pw).view(np.float32)  # od0
        hdr = np.broadcast_to(w, (C, 8)).copy()
        in_maps.append({"hdr": hdr, "xh": xh_all[i]})
    return in_maps


def run(x, p, **spmd_kwargs):
    from concourse.bass_utils import run_bass_kernel_spmd

    nc = _get_bass()
    in_maps = _make_in_maps(x, p)
    res = run_bass_kernel_spmd(nc, in_maps, core_ids=list(range(N_CORES)), **spmd_kwargs)
    out = np.stack(
        [
            res.results[i]["out"].astype(np.float32).reshape(OC, OH, OW)
            for i in range(B)
        ],
        axis=0,
    )
    return out, res


def kernel(x, p):
    out, _ = run(x, p)
    return out


# revision 5
# speedup vs baseline: 1.1336x; 1.1336x over previous
"""Trainium2 Bass kernel v7 = v6 + prewarm/memset fixes from v6's trace.

v6 measured 19.4 us. Its trace showed the ACT Copy-table prewarm was
gated on the 3.5 us ebuf memset (s_m), putting the 1.28 us table load
back on the critical path, and the memset itself (no fast mode at fp16)
delayed DVE's first rot past input-arrival. Fixes: dummy gets its own
semaphore so the table loads at ~7 us during the input DMA, and the
memset runs on a bitcast-fp32 view (half the elements).

v5's NTFF trace showed the output was HWDGE descriptor-generation bound:
4096 row-descriptors at ~1.65 ns each ≈ 7 us on one ring, with per-packet
floors ~10-17 ns swamping the byte savings of fp16.

v6 kills the descriptor count: with fp16 output, each written channel
(2c+r) is ONE contiguous span. The key identity: in a row-major image,
shifting the flat span start by ph*64 + pw shifts rows by ph AND columns
by pw at once. So the kernel writes E'[c] - a fixed 64x64 fp16 image with
values at even (row, col) positions, zeros elsewhere (pre-memset) - and
the whole per-sample (r, ph, pw) selection is a single dynamic flat
offset od = r*4096 + ph*64 + pw on the (c, two*4096) view. Span lengths
are tightened to 128k-65 so the largest end index is exactly 8191 (last
nonzero element), and inter-chunk/channel gaps land on positions that are
zero by the runtime's zero-init contract.

Per chunk (k a-rows): ACT rot (k*32 elems), DVE fused select (k*32), one
output DMA of 128 descriptors (~2 KiB each). Total 512 descriptors vs
v5's 4096.
"""

import numpy as np

B, C, H, W = 8, 128, 32, 32
OC, OH, OW = 2 * C, 2 * H, 2 * W
N_CORES = 8

CHUNKS = (4, 8, 10, 10)
_compiled = {}


def _build_bass(chunks=None):
    from contextlib import ExitStack

    import concourse.bass as bass
    import concourse.mybir as mybir

    chunks = chunks or CHUNKS

    fp32 = mybir.dt.float32
    fp16 = mybir.dt.float16
    orig_aeb = bass.Bass.all_engine_barrier
    bass.Bass.all_engine_barrier = lambda self, **kw: None
    try:
        nc = bass.Bass(enable_partition_id=False)
    finally:
        bass.Bass.all_engine_barrier = orig_aeb

    Copy = mybir.ActivationFunctionType.Copy
    SP = mybir.EngineType.SP

    hdr = nc.dram_tensor("hdr", (C, 8), fp32, kind="ExternalInput")
    xh = nc.dram_tensor("xh", (C, H * W), fp16, kind="ExternalInput")
    out = nc.dram_tensor("out", (OC, OH * OW), fp16, kind="ExternalOutput")

    ctx = ExitStack()
    with ctx:
        hbuf = ctx.enter_context(nc.sbuf_tensor([C, 8], fp32))
        xbuf = ctx.enter_context(nc.sbuf_tensor([C, H * W], fp16))
        ubuf = ctx.enter_context(nc.sbuf_tensor([C, H * W], fp16))
        ebuf = ctx.enter_context(nc.sbuf_tensor([C, OH * OW], fp16))  # 64x64
        dummy = ctx.enter_context(nc.sbuf_tensor([C, 2], fp32))

        s_d = nc.alloc_semaphore("s_d")  # dummy ready (table prewarm)
        s_h = nc.alloc_semaphore("s_h")
        s_x = nc.alloc_semaphore("s_x")
        s_m = nc.alloc_semaphore("s_m")
        s_r0 = nc.alloc_semaphore("s_r0")
        s_r = nc.alloc_semaphore("s_r")
        s_v = nc.alloc_semaphore("s_v")
        s_out = nc.alloc_semaphore("s_out")

        x3 = xbuf[:].rearrange("p (a b) -> p a b", b=W)
        u3 = ubuf[:].rearrange("p (a b) -> p a b", b=W)
        e3 = ebuf[:].rearrange("p (oh ow) -> p oh ow", ow=OW)  # (c, 64, 64)
        # channel pair (2c, 2c+1) flattened: od = r*4096 + ph*64 + pw picks
        # the start; spans tile exactly at 1024-elem strides per chunk
        outv = out[:].rearrange("(c two) m -> c (two m)", two=2)  # (128, 8192)
        wm = hbuf[:, 0:2]  # [m1, m0]
        qbits = hbuf[:, 2:8].bitcast(mybir.dt.int32)

        # ---- T0: x halves on both rings; header behind x on ACT ----
        HWH = H * W // 2
        nc.sync.dma_start(xbuf[:, 0:HWH], xh[:, 0:HWH]).then_inc(s_x, 16)
        nc.scalar.dma_start(xbuf[:, HWH:], xh[:, HWH:]).then_inc(s_x, 16)
        nc.scalar.dma_start(hbuf[:], hdr[:]).then_inc(s_h, 16)

        # ---- T0: DVE zero-fills E' + dummy; ACT prewarms Copy table ----
        nc.vector.memset(dummy[:, :], 0.0).then_inc(s_d, 1)
        nc.vector.memset(ebuf[:].bitcast(fp32), 0.0).then_inc(s_m, 1)
        nc.scalar.wait_ge(s_d, 1)
        nc.scalar.activation(dummy[:, 1:2], dummy[:, 0:1], Copy, scale=0.0)

        # ---- SP: one offset register, pre-dispatch output DMAs ----
        nc.sync.wait_ge(s_h, 16)
        od0 = nc.values_load(
            qbits[0:1, 0:1],
            engines=[SP],
            min_val=0,
            max_val=OH * OW + OW + 1,  # 4161
            skip_runtime_bounds_check=True,
        )
        nc.sync.wait_ge(s_m, 1)
        a0 = 0
        for j, n in enumerate(chunks):
            odj = od0 if j == 0 else nc.sync.compute_val(od0 + 128 * a0)
            span = 128 * n - 65  # through the last nonzero element
            nc.sync.dma_start(
                outv[:, bass.ds(odj, span)],
                ebuf[:, 128 * a0 : 128 * a0 + span],
            )._wait_ge(s_v, j + 1).then_inc(s_out, 16)
            a0 += n

        # ---- DVE: chunk-0 rot locally, then fused select per chunk ----
        nc.vector.wait_ge(s_h, 16)
        nc.vector.wait_ge(s_x, 32)
        n0 = chunks[0]
        nc.vector.tensor_scalar_mul(
            u3[:, 0:1, :], x3[:, :, 0:1].transpose([0, 2, 1]), wm[:, 0:1]
        )
        nc.vector.tensor_scalar_mul(
            u3[:, 1:n0, :],
            x3[:, :, W - 1 : W - n0 : -1].transpose([0, 2, 1]),
            wm[:, 0:1],
        ).then_inc(s_r0, 1)

        # ---- ACT: rots for chunks 1.. ----
        nc.scalar.wait_ge(s_h, 16)
        nc.scalar.wait_ge(s_x, 32)
        a0 = n0
        for j, n in enumerate(chunks[1:], start=1):
            nc.scalar.activation(
                u3[:, a0 : a0 + n, :],
                x3[:, :, W - a0 : W - a0 - n : -1].transpose([0, 2, 1]),
                Copy,
                scale=wm[:, 0:1],
            ).then_inc(s_r, 1)
            a0 += n

        a0 = 0
        for j, n in enumerate(chunks):
            nc.vector.wait_ge(s_r0, 1) if j == 0 else nc.vector.wait_ge(s_r, j)
            nc.vector.scalar_tensor_tensor(
                e3[:, 2 * a0 : 2 * (a0 + n) : 2, 0::2],
                x3[:, a0 : a0 + n, :],
                wm[:, 1:2],
                u3[:, a0 : a0 + n, :],
                op0=mybir.AluOpType.mult,
                op1=mybir.AluOpType.add,
            ).then_inc(s_v, 1)
            a0 += n

        nc.sync.wait_ge(s_out, 16 * len(chunks))
    return nc


def _get_bass():
    if "nc" not in _compiled:
        _compiled["nc"] = _build_bass()
    return _compiled["nc"]


def _make_in_maps(x, p, chunks=None):
    x = np.asarray(x, dtype=np.float32)
    p = np.asarray(p)
    xh_all = x.reshape(B, C, H * W).astype(np.float16)
    in_maps = []
    for i in range(B):
        ph, pw, r = int(p[i, 0]), int(p[i, 1]), int(p[i, 2])
        assert r in (0, 1) and ph in (0, 1) and pw in (0, 1)
        w = np.zeros(8, np.float32)
        w[0] = 1.0 if r == 1 else 0.0  # m1
        w[1] = 1.0 if r == 0 else 0.0  # m0
        w[2] = np.int32(r * 4096 + ph * 64 + pw).view(np.float32)  # od0
        hdr = np.broadcast_to(w, (C, 8)).copy()
        in_maps.append({"hdr": hdr, "xh": xh_all[i]})
    return in_maps


def run(x, p, **spmd_kwargs):
    from concourse.bass_utils import run_bass_kernel_spmd

    nc = _get_bass()
    in_maps = _make_in_maps(x, p)
    res = run_bass_kernel_spmd(nc, in_maps, core_ids=list(range(N_CORES)), **spmd_kwargs)
    out = np.stack(
        [
            res.results[i]["out"].astype(np.float32).reshape(OC, OH, OW)
            for i in range(B)
        ],
        axis=0,
    )
    return out, res


def kernel(x, p):
    out, _ = run(x, p)
    return out
